# revision 1
# baseline (speedup 1.0000x reference)
"""FFF (fast feedforward / MoE-routing binary tree) forward pass on 8 Trainium2 NeuronCores.

Strategy (data-parallel over the 16384-token batch, 2048 tokens/core):
  - Levels 0..7 (255 nodes) are computed DENSE: logits via PE fp32 matmul,
    tree walk via one-hot map maintenance on DVE, masked acts @ w_out.T via PE.
  - Levels 8..11 (3840 nodes) are computed SPARSE: each token only needs one
    node per level, so we gather w_in rows by the walked node index
    (indirect DMA), form the logit with a fused multiply-reduce on DVE, and
    accumulate coef * w_outT[idx] into the same PSUM banks via a diagonal
    fp32 matmul on PE.
  Host pre-transposes x tiles / shallow weights so no on-device transposes of
  inputs are needed (PE only transposes the 255-wide masked activations).
"""

import numpy as np

P = 128
D = 1024
KC = 8                 # 1024 / 128 contraction chunks
N_NODES = 4095
SH_NODES = 255         # nodes in levels 0..7
SHN = 256              # padded
DEPTH = 11
N_CORES = 8
TOK = 2048             # tokens per core
NT = TOK // P          # 16 token tiles per core


def build_nc():
    import os
    from concourse import bacc, bass, mybir, tile
    from concourse.masks import make_identity

    stage = os.environ.get("KERNEL_STAGE", "full")
    deep_on = stage not in ("shallow",)
    batch_gather = stage in ("batchgather",)  # (128,4)-idx gather is broken on HW
    deep_mm_on = stage not in ("nodeepmm",)
    debug_dump = os.environ.get("KERNEL_DEBUG", "0") == "1"

    dt = mybir.dt
    AFT = mybir.ActivationFunctionType
    ALU = mybir.AluOpType

    nc = bacc.Bacc("TRN2", target_bir_lowering=False, debug=False)

    x_d = nc.dram_tensor("x", [TOK, D], dt.float32, kind="ExternalInput")
    xT_d = nc.dram_tensor("xT", [NT, KC, P, P], dt.float32, kind="ExternalInput")
    # wcat[n] = [w_in[n, :], w_outT[n, :]] — one 8KB gather serves both the
    # deep logit dot and the deep output accumulation.
    wcat_d = nc.dram_tensor("wcat", [N_NODES, 2 * D], dt.float32, kind="ExternalInput")
    w_inT_sh_d = nc.dram_tensor("w_inT_sh", [KC, P, SHN], dt.float32, kind="ExternalInput")
    woT_sh_d = nc.dram_tensor("woT_sh", [2, P, D], dt.float32, kind="ExternalInput")
    out_d = nc.dram_tensor("out", [TOK, D], dt.float32, kind="ExternalOutput")
    dbg = {}

    if debug_dump:
        dbg["logits"] = nc.dram_tensor("dbg_logits", [NT, P, SHN], dt.float32, kind="ExternalOutput")
        dbg["map"] = nc.dram_tensor("dbg_map", [NT, P, SHN], dt.float32, kind="ExternalOutput")
        dbg["mskT"] = nc.dram_tensor("dbg_mskT", [NT, P, 2 * P], dt.float32, kind="ExternalOutput")
        dbg["idx"] = nc.dram_tensor("dbg_idx", [NT, P, 4], dt.int32, kind="ExternalOutput")
        dbg["coef"] = nc.dram_tensor("dbg_coef", [NT, P, 4], dt.float32, kind="ExternalOutput")

    with tile.TileContext(nc) as tc:
        with (
            tc.tile_pool(name="const", bufs=1) as cpool,
            tc.tile_pool(name="xT", bufs=2) as xT_pool,
            tc.tile_pool(name="xn", bufs=5) as xn_pool,
            tc.tile_pool(name="small", bufs=4) as small_pool,
            tc.tile_pool(name="tiny", bufs=8) as tiny_pool,
            tc.tile_pool(name="mskT", bufs=4) as mskT_pool,
            tc.tile_pool(name="win", bufs=10) as win_pool,
            tc.tile_pool(name="dscr", bufs=2) as dscr_pool,
            tc.tile_pool(name="osb", bufs=3) as osb_pool,
            tc.tile_pool(name="lps", bufs=2, space="PSUM") as lps_pool,
            tc.tile_pool(name="tps", bufs=2, space="PSUM") as tps_pool,
            tc.tile_pool(name="ops", bufs=4, space="PSUM") as ops_pool,
        ):
            ident = cpool.tile([P, P], dt.float32)
            make_identity(nc, ident[:])
            w_inT_sb = cpool.tile([P, KC * SHN], dt.float32)
            nc.sync.dma_start(
                out=w_inT_sb[:].rearrange("p (k n) -> p k n", k=KC),
                in_=w_inT_sh_d[:].rearrange("k p n -> p k n"),
            )
            woT_sb = cpool.tile([P, 2 * D], dt.float32)
            nc.sync.dma_start(
                out=woT_sb[:].rearrange("p (c o) -> p c o", c=2),
                in_=woT_sh_d[:].rearrange("c p o -> p c o"),
            )

            for t in range(NT):
                xT = xT_pool.tile([P, D], dt.float32)
                nc.sync.dma_start(
                    out=xT[:].rearrange("p (k j) -> p k j", k=KC),
                    in_=xT_d[t].rearrange("k p j -> p k j"),
                )
                xn = xn_pool.tile([P, D], dt.float32)
                nc.sync.dma_start(out=xn[:], in_=x_d[t * P:(t + 1) * P, :])

                # ---- dense shallow logits: (128 tokens, 256 nodes) ----
                lps = lps_pool.tile([P, SHN], dt.float32, space="PSUM")
                for k in range(KC):
                    nc.tensor.matmul(
                        out=lps[:],
                        lhsT=xT[:, k * P:(k + 1) * P],
                        rhs=w_inT_sb[:, k * SHN:(k + 1) * SHN],
                        start=(k == 0),
                        stop=(k == KC - 1),
                    )
                lsb = small_pool.tile([P, SHN], dt.float32, tag="lsb")
                nc.scalar.copy(out=lsb[:], in_=lps[:])
                acts = small_pool.tile([P, SHN], dt.float32, tag="acts")
                nc.scalar.activation(out=acts[:], in_=lps[:], func=AFT.Gelu)

                # ---- shallow walk: one-hot decision map + heap index r ----
                mp = small_pool.tile([P, SHN], dt.float32, tag="map")
                nc.vector.memset(mp[:], 0.0)
                nc.vector.memset(mp[:, 0:1], 1.0)
                r = tiny_pool.tile([P, 1], dt.float32, tag="r")
                s2 = tiny_pool.tile([P, 1], dt.float32, tag="s2")
                pick = tiny_pool.tile([P, 1], dt.float32, tag="pick")
                dec = tiny_pool.tile([P, P], dt.float32, tag="dec")
                scr = tiny_pool.tile([P, P], dt.float32, tag="scr")
                # level 0: map[1]=1-dec0, map[2]=dec0, r=2+dec0
                nc.vector.tensor_scalar(
                    out=mp[:, 2:3], in0=lsb[:, 0:1], scalar1=0.0, scalar2=None, op0=ALU.is_gt
                )
                nc.vector.tensor_scalar(
                    out=mp[:, 1:2], in0=lsb[:, 0:1], scalar1=0.0, scalar2=None, op0=ALU.is_le
                )
                nc.vector.tensor_scalar(
                    out=r[:], in0=mp[:, 2:3], scalar1=2.0, scalar2=None, op0=ALU.add
                )
                for d in range(1, 8):
                    o = 2 ** d - 1
                    w = 2 ** d
                    nc.vector.tensor_scalar(
                        out=dec[:, :w], in0=lsb[:, o:o + w],
                        scalar1=0.0, scalar2=None, op0=ALU.is_gt,
                    )
                    nc.vector.tensor_scalar(
                        out=s2[:], in0=r[:], scalar1=2.0, scalar2=None, op0=ALU.mult
                    )
                    if d < 7:
                        o1 = 2 ** (d + 1) - 1
                        nxt = mp[:, o1:o1 + 2 * w].rearrange("p (n two) -> p n two", two=2)
                        # odd slots = OH*dec
                        nc.vector.tensor_tensor(
                            out=nxt[:, :, 1], in0=mp[:, o:o + w], in1=dec[:, :w],
                            op=ALU.mult,
                        )
                        nc.vector.tensor_reduce(
                            out=pick[:], in_=nxt[:, :, 1],
                            axis=mybir.AxisListType.X, op=ALU.add,
                        )
                        # even slots = OH - odd
                        nc.vector.tensor_tensor(
                            out=nxt[:, :, 0], in0=mp[:, o:o + w], in1=nxt[:, :, 1],
                            op=ALU.subtract,
                        )
                    else:
                        nc.vector.tensor_tensor(
                            out=scr[:, :w], in0=mp[:, o:o + w], in1=dec[:, :w],
                            op=ALU.mult,
                        )
                        nc.vector.tensor_reduce(
                            out=pick[:], in_=scr[:, :w],
                            axis=mybir.AxisListType.X, op=ALU.add,
                        )
                    nc.vector.tensor_tensor(out=r[:], in0=s2[:], in1=pick[:], op=ALU.add)

                # ---- masked acts + transpose for mm2 ----
                msk = small_pool.tile([P, SHN], dt.float32, tag="msk")
                nc.vector.tensor_tensor(out=msk[:], in0=acts[:], in1=mp[:], op=ALU.mult)
                mskT = mskT_pool.tile([P, 2 * P], dt.float32)
                for c in range(2):
                    tp = tps_pool.tile([P, P], dt.float32, space="PSUM")
                    nc.tensor.transpose(
                        out=tp[:], in_=msk[:, c * P:(c + 1) * P], identity=ident[:]
                    )
                    nc.scalar.copy(out=mskT[:, c * P:(c + 1) * P], in_=tp[:])

                # ---- deep levels 8..11: gather + fused dot + walk ----
                coef4 = tiny_pool.tile([P, 4], dt.float32, tag="coef4")
                idx4 = tiny_pool.tile([P, 4], dt.int32, tag="idx4")
                idxf = tiny_pool.tile([P, 1], dt.float32, tag="idxf")
                logit = tiny_pool.tile([P, 1], dt.float32, tag="logit")
                dscr = dscr_pool.tile([P, D], dt.float32)
                gws = []
                for l in range(4 if deep_on else 0):
                    dlev = 8 + l
                    nc.vector.tensor_scalar(
                        out=idxf[:], in0=r[:], scalar1=-1.0, scalar2=None, op0=ALU.add
                    )
                    nc.vector.tensor_copy(out=idx4[:, l:l + 1], in_=idxf[:])
                    gw = win_pool.tile([P, 2 * D], dt.float32)
                    gws.append(gw)
                    nc.gpsimd.indirect_dma_start(
                        out=gw[:],
                        out_offset=None,
                        in_=wcat_d[:],
                        in_offset=bass.IndirectOffsetOnAxis(ap=idx4[:, l:l + 1], axis=0),
                    )
                    nc.vector.tensor_tensor(
                        out=dscr[:], in0=xn[:], in1=gw[:, 0:D], op=ALU.mult
                    )
                    nc.vector.tensor_reduce(
                        out=logit[:], in_=dscr[:],
                        axis=mybir.AxisListType.X, op=ALU.add,
                    )
                    nc.scalar.activation(out=coef4[:, l:l + 1], in_=logit[:], func=AFT.Gelu)
                    if dlev < DEPTH:
                        nc.vector.tensor_scalar(
                            out=dec[:, 0:1], in0=logit[:], scalar1=0.0, scalar2=None,
                            op0=ALU.is_gt,
                        )
                        nc.vector.tensor_scalar(
                            out=s2[:], in0=r[:], scalar1=2.0, scalar2=None, op0=ALU.mult
                        )
                        nc.vector.tensor_tensor(
                            out=r[:], in0=s2[:], in1=dec[:, 0:1], op=ALU.add
                        )

                # ---- output accumulation in PSUM (one half-bank per N-half,
                # emitted after the deep chain so the bank is held briefly) ----
                dmm = deep_on and deep_mm_on
                diags = []
                for l in range(4 if dmm else 0):
                    dg = tiny_pool.tile([P, P], dt.float32, tag="diag")
                    nc.vector.tensor_scalar(
                        out=dg[:], in0=ident[:], scalar1=coef4[:, l:l + 1],
                        scalar2=None, op0=ALU.mult,
                    )
                    diags.append(dg)
                osb = osb_pool.tile([P, D], dt.float32)
                for h in range(2):
                    ops = ops_pool.tile([P, 512], dt.float32, space="PSUM")
                    for c in range(2):
                        nc.tensor.matmul(
                            out=ops[:],
                            lhsT=mskT[:, c * P:(c + 1) * P],
                            rhs=woT_sb[:, c * D + h * 512: c * D + h * 512 + 512],
                            start=(c == 0),
                            stop=(c == 1 and not dmm),
                            skip_group_check=True,
                        )
                    for l in range(4 if dmm else 0):
                        nc.tensor.matmul(
                            out=ops[:],
                            lhsT=diags[l][:],
                            rhs=gws[l][:, D + h * 512: D + h * 512 + 512],
                            start=False,
                            stop=(l == 3),
                            skip_group_check=True,
                        )
                    nc.scalar.copy(out=osb[:, h * 512:(h + 1) * 512], in_=ops[:])
                nc.sync.dma_start(out=out_d[t * P:(t + 1) * P, :], in_=osb[:])

                if debug_dump:
                    nc.sync.dma_start(out=dbg["logits"][t], in_=lsb[:])
                    nc.sync.dma_start(out=dbg["map"][t], in_=mp[:])
                    nc.sync.dma_start(out=dbg["mskT"][t], in_=mskT[:])
                    nc.sync.dma_start(out=dbg["idx"][t], in_=idx4[:])
                    nc.sync.dma_start(out=dbg["coef"][t], in_=coef4[:])

    nc.compile()
    return nc


def host_prep(x, w_in, w_out):
    """Build the per-core input maps (host-side transposes/tilings)."""
    x = np.ascontiguousarray(x, np.float32)
    w_in = np.ascontiguousarray(w_in, np.float32)
    w_out = np.ascontiguousarray(w_out, np.float32)

    w_inT_sh = np.zeros((SHN, D), np.float32)
    w_inT_sh[:SH_NODES] = w_in[:SH_NODES]
    w_inT_sh = np.ascontiguousarray(
        w_inT_sh.T.reshape(KC, P, SHN)
    )  # [k,p,n] = w_in[n, k*128+p]

    woT_sh = np.zeros((SHN, D), np.float32)
    woT_sh[:SH_NODES] = w_out[:, :SH_NODES].T
    woT_sh = np.ascontiguousarray(woT_sh.reshape(2, P, D))  # [c,p,o] = w_out[o, c*128+p]

    wcat = np.ascontiguousarray(
        np.concatenate([w_in, w_out.T], axis=1)
    )  # (4095, 2048): [w_in row | w_outT row]

    in_maps = []
    for c in range(N_CORES):
        xs = x[c * TOK:(c + 1) * TOK]
        xT = np.ascontiguousarray(
            xs.reshape(NT, P, KC, P).transpose(0, 2, 3, 1)
        )  # [t,k,p,j] = xs[t*128+j, k*128+p]
        in_maps.append(
            {
                "x": np.ascontiguousarray(xs),
                "xT": xT,
                "wcat": wcat,
                "w_inT_sh": w_inT_sh,
                "woT_sh": woT_sh,
            }
        )
    return in_maps


_NC_CACHE = {}


def kernel(x, w_in, w_out, force_depth=None, **_ignored):
    from concourse.bass_utils import run_bass_kernel_spmd

    if "nc" not in _NC_CACHE:
        _NC_CACHE["nc"] = build_nc()
    nc = _NC_CACHE["nc"]

    in_maps = host_prep(np.asarray(x), np.asarray(w_in), np.asarray(w_out))
    res = run_bass_kernel_spmd(nc, in_maps, core_ids=list(range(N_CORES)))
    out = np.concatenate([res.results[c]["out"] for c in range(N_CORES)], axis=0)
    return out.astype(np.float32)


if __name__ == "__main__":
    import reference

    inputs = reference.setup_inputs()
    expected = np.asarray(reference.reference(**inputs))
    actual = kernel(**{k: np.asarray(v) for k, v in inputs.items()})
    err = np.abs(actual - expected).max()
    print("absmax err:", err)



# revision 16
# speedup vs baseline: 1.0886x; 1.0886x over previous
"""FFF (fast feedforward / MoE-routing binary tree) forward pass on 8 Trainium2 NeuronCores.

v2 — level-major deep phase for DMA overlap.

Strategy (data-parallel over the 16384-token batch, 2048 tokens/core):
  - Levels 0..7 (255 nodes) dense: fp32 PE matmul logits; gelu acts + decision
    bits computed straight from PSUM; the binary-tree walk is FUSED across all
    16 token tiles (one DVE op per level instead of 16).
  - Levels 8..11 sparse, LEVEL-MAJOR: for each level, gather wcat rows for all
    16 tiles (indirect DMA pipelines across tiles on the DMA rings), fused
    dot (tensor_tensor_reduce) per tile, per-block-of-4 index update.  This
    removes the per-tile serial gather->dot->gather chain of v1.
  - Output matmul runs in bf16 (tolerance is ~2e-2; routing stays fp32-exact).
    Deep contributions accumulate into a per-tile bf16 tensor on DVE, then get
    added into the PSUM accumulation group via an identity matmul.
  - Gather rows are 6KB: [w_in row f32 | w_outT row bf16 packed as f32 words].
"""

import numpy as np

P = 128
D = 1024
KC = 8                  # 1024 / 128 contraction chunks
N_NODES = 4095
SH_NODES = 255          # nodes in levels 0..7
SHN = 256               # padded
DEPTH = 11
N_CORES = 8
TOK = 2048              # tokens per core
NT = TOK // P           # 16 token tiles per core
NB = 4                  # deep-phase blocks
BT = NT // NB           # tiles per block
WB = D + D // 2         # wcat row in f32 words: 1024 f32 w_in + 512 packed bf16 w_out
NDL = 4                 # deep levels (8..11)


def build_nc():
    import os
    from concourse import bacc, bass, mybir, tile
    from concourse.masks import make_identity

    stage = os.environ.get("KERNEL_STAGE", "full")
    deep_on = stage not in ("shallow",)
    do_dots = stage in ("dots", "dacc", "full")
    do_dacc = stage in ("dacc", "full")
    do_identmm = stage == "full"
    debug_dump = os.environ.get("KERNEL_DEBUG", "0") == "1"

    dt = mybir.dt
    AFT = mybir.ActivationFunctionType
    ALU = mybir.AluOpType
    AXL = mybir.AxisListType

    nc = bacc.Bacc("TRN2", target_bir_lowering=False, debug=False)

    x_d = nc.dram_tensor("x", [TOK, D], dt.float32, kind="ExternalInput")
    xT_d = nc.dram_tensor("xT", [NT, KC, P, P], dt.float32, kind="ExternalInput")
    # wcat[n] = [w_in[n, :] f32 | w_outT[n, :] bf16 packed in f32 words]
    wcat_d = nc.dram_tensor("wcat", [N_NODES, WB], dt.float32, kind="ExternalInput")
    w_inT_sh_d = nc.dram_tensor("w_inT_sh", [KC, P, SHN], dt.float32, kind="ExternalInput")
    woT_bf_d = nc.dram_tensor("woT_bf", [2, P, D], dt.bfloat16, kind="ExternalInput")
    out_d = nc.dram_tensor("out", [TOK, D], dt.float32, kind="ExternalOutput")
    dbg = {}
    if debug_dump:
        dbg["r"] = nc.dram_tensor("dbg_r", [P, NT], dt.float32, kind="ExternalOutput")
        dbg["map"] = nc.dram_tensor("dbg_map", [P, NT * SHN], dt.bfloat16, kind="ExternalOutput")
        dbg["dec"] = nc.dram_tensor("dbg_dec", [P, NT * SHN], dt.bfloat16, kind="ExternalOutput")
        dbg["acts"] = nc.dram_tensor("dbg_acts", [P, NT * SHN], dt.bfloat16, kind="ExternalOutput")
        dbg["logit"] = nc.dram_tensor("dbg_logit", [NDL, NB, P, BT], dt.float32, kind="ExternalOutput")
        dbg["idx"] = nc.dram_tensor("dbg_idx", [NDL, NB, P, BT], dt.int32, kind="ExternalOutput")

    with tile.TileContext(nc) as tc:
        with (
            tc.tile_pool(name="const", bufs=1) as cpool,
            tc.tile_pool(name="xTp", bufs=2) as xT_pool,
            tc.tile_pool(name="gwp", bufs=int(os.environ.get("GW_BUFS", "5"))) as gw_pool,
            tc.tile_pool(name="daccp", bufs=NT) as dacc_pool,
            tc.tile_pool(name="mskTp", bufs=6) as mskT_pool,
            tc.tile_pool(name="scrp", bufs=2) as scr_pool,
            tc.tile_pool(name="osbp", bufs=3) as osb_pool,
            tc.tile_pool(name="tinyp", bufs=4) as tiny_pool,
            tc.tile_pool(name="lpsp", bufs=2, space="PSUM") as lps_pool,
            tc.tile_pool(name="tpsp", bufs=2, space="PSUM") as tps_pool,
            tc.tile_pool(name="opsp", bufs=4, space="PSUM") as ops_pool,
        ):
            ident = cpool.tile([P, P], dt.bfloat16)
            make_identity(nc, ident[:])
            w_inT_sb = cpool.tile([P, KC * SHN], dt.float32)
            nc.sync.dma_start(
                out=w_inT_sb[:].rearrange("p (k n) -> p k n", k=KC),
                in_=w_inT_sh_d[:].rearrange("k p n -> p k n"),
            )
            woT_sb = cpool.tile([P, 2 * D], dt.bfloat16)
            nc.sync.dma_start(
                out=woT_sb[:].rearrange("p (c o) -> p c o", c=2),
                in_=woT_bf_d[:].rearrange("c p o -> p c o"),
            )

            # x in natural layout, all 16 tiles resident (deep dots need it 4x)
            xn_all = cpool.tile([P, NT * D], dt.float32)
            for t in range(NT):
                nc.sync.dma_start(
                    out=xn_all[:, t * D:(t + 1) * D], in_=x_d[t * P:(t + 1) * P, :]
                )

            dec_all = cpool.tile([P, NT * SHN], dt.bfloat16)
            acts_all = cpool.tile([P, NT * SHN], dt.bfloat16)
            map_all = cpool.tile([P, NT * SHN], dt.bfloat16)

            # ---- dense shallow logits burst (PE fp32) ----
            for t in range(NT):
                xT = xT_pool.tile([P, D], dt.float32)
                nc.sync.dma_start(
                    out=xT[:].rearrange("p (k j) -> p k j", k=KC),
                    in_=xT_d[t].rearrange("k p j -> p k j"),
                )
                lps = lps_pool.tile([P, SHN], dt.float32, space="PSUM")
                for k in range(KC):
                    nc.tensor.matmul(
                        out=lps[:],
                        lhsT=xT[:, k * P:(k + 1) * P],
                        rhs=w_inT_sb[:, k * SHN:(k + 1) * SHN],
                        start=(k == 0),
                        stop=(k == KC - 1),
                    )
                nc.scalar.activation(
                    out=acts_all[:, t * SHN:(t + 1) * SHN], in_=lps[:], func=AFT.Gelu
                )
                nc.vector.tensor_scalar(
                    out=dec_all[:, t * SHN:(t + 1) * SHN], in0=lps[:],
                    scalar1=0.0, scalar2=None, op0=ALU.is_gt,
                )

            # ---- fused walk across all 16 tiles ----
            dec3 = dec_all[:].rearrange("p (t n) -> p t n", t=NT)
            map3 = map_all[:].rearrange("p (t n) -> p t n", t=NT)
            r_all = cpool.tile([P, NT], dt.float32)      # 1-based heap index
            pick = cpool.tile([P, NT], dt.float32)
            wscr = cpool.tile([P, NT * P], dt.bfloat16)  # level-7 odd scratch

            nc.vector.memset(map_all[:], 0.0)
            nc.vector.memset(map3[:, :, 0:1], 1.0)
            # level 0: map[1] = 1-dec0, map[2] = dec0, r = 2+dec0
            nc.vector.tensor_copy(out=map3[:, :, 2:3], in_=dec3[:, :, 0:1])
            nc.vector.tensor_scalar(
                out=map3[:, :, 1:2], in0=dec3[:, :, 0:1],
                scalar1=-1.0, scalar2=1.0, op0=ALU.mult, op1=ALU.add,
            )
            nc.vector.tensor_scalar(
                out=r_all[:], in0=dec3[:, :, 0:1], scalar1=2.0, scalar2=None,
                op0=ALU.add,
            )
            for d in range(1, 8):
                o = 2 ** d - 1
                w = 2 ** d
                if d < 7:
                    o1 = 2 ** (d + 1) - 1
                    nxt = map3[:, :, o1:o1 + 2 * w].rearrange(
                        "p t (n two) -> p t n two", two=2
                    )
                    # odd slots = OH*dec
                    nc.vector.tensor_tensor(
                        out=nxt[:, :, :, 1], in0=map3[:, :, o:o + w],
                        in1=dec3[:, :, o:o + w], op=ALU.mult,
                    )
                    nc.vector.tensor_reduce(
                        out=pick[:], in_=nxt[:, :, :, 1], axis=AXL.X, op=ALU.add,
                    )
                    # even slots = OH - odd
                    nc.vector.tensor_tensor(
                        out=nxt[:, :, :, 0], in0=map3[:, :, o:o + w],
                        in1=nxt[:, :, :, 1], op=ALU.subtract,
                    )
                else:
                    ws3 = wscr[:].rearrange("p (t n) -> p t n", t=NT)
                    nc.vector.tensor_tensor(
                        out=ws3[:, :, :w], in0=map3[:, :, o:o + w],
                        in1=dec3[:, :, o:o + w], op=ALU.mult,
                    )
                    nc.vector.tensor_reduce(
                        out=pick[:], in_=ws3[:, :, :w], axis=AXL.X, op=ALU.add,
                    )
                nc.vector.scalar_tensor_tensor(
                    out=r_all[:], in0=r_all[:], scalar=2.0, in1=pick[:],
                    op0=ALU.mult, op1=ALU.add,
                )

            if debug_dump:
                nc.sync.dma_start(out=dbg["r"][:], in_=r_all[:])
                nc.sync.dma_start(out=dbg["map"][:], in_=map_all[:])
                nc.sync.dma_start(out=dbg["dec"][:], in_=dec_all[:])
                nc.sync.dma_start(out=dbg["acts"][:], in_=acts_all[:])

            # ---- masked acts (in place) + per-tile transposes ----
            nc.vector.tensor_tensor(
                out=acts_all[:], in0=acts_all[:], in1=map_all[:], op=ALU.mult
            )
            mskTs = []
            for t in range(NT):
                mskT = mskT_pool.tile([P, 2 * P], dt.bfloat16)
                mskTs.append(mskT)
                for c in range(2):
                    tp = tps_pool.tile([P, P], dt.bfloat16, space="PSUM")
                    nc.tensor.transpose(
                        out=tp[:],
                        in_=acts_all[:, t * SHN + c * P: t * SHN + (c + 1) * P],
                        identity=ident[:],
                    )
                    nc.scalar.copy(out=mskT[:, c * P:(c + 1) * P], in_=tp[:])

            # ---- deep levels 8..11, level-major in blocks of 4 tiles ----
            daccs = [None] * NT
            if deep_on:
                rbs = []
                for b in range(NB):
                    rb = tiny_pool.tile([P, BT], dt.float32, tag="rb", bufs=NB)
                    nc.vector.tensor_copy(out=rb[:], in_=r_all[:, b * BT:(b + 1) * BT])
                    rbs.append(rb)
                for l in range(NDL):
                    for b in range(NB):
                        rb = rbs[b]
                        idxf = tiny_pool.tile([P, BT], dt.float32, tag="idxf")
                        nc.vector.tensor_scalar(
                            out=idxf[:], in0=rb[:], scalar1=-1.0, scalar2=None,
                            op0=ALU.add,
                        )
                        idxi = tiny_pool.tile([P, BT], dt.int32, tag="idxi")
                        nc.vector.tensor_copy(out=idxi[:], in_=idxf[:])
                        if debug_dump:
                            nc.sync.dma_start(out=dbg["idx"][l, b], in_=idxi[:])
                        gws = []
                        for ti in range(BT):
                            gw = gw_pool.tile([P, WB], dt.float32)
                            nc.gpsimd.indirect_dma_start(
                                out=gw[:],
                                out_offset=None,
                                in_=wcat_d[:],
                                in_offset=bass.IndirectOffsetOnAxis(
                                    ap=idxi[:, ti:ti + 1], axis=0
                                ),
                                bounds_check=N_NODES - 1,
                                oob_is_err=False,
                            )
                            gws.append(gw)
                        if do_dots:
                            logit_b = tiny_pool.tile([P, BT], dt.float32, tag="logit_b")
                            use_ttr = os.environ.get("USE_TTR", "0") == "1"
                            for ti, gw in enumerate(gws):
                                t = b * BT + ti
                                scr = scr_pool.tile([P, D], dt.float32)
                                if use_ttr:
                                    nc.vector.tensor_tensor_reduce(
                                        out=scr[:],
                                        in0=xn_all[:, t * D:(t + 1) * D],
                                        in1=gw[:, 0:D],
                                        scale=1.0, scalar=0.0,
                                        op0=ALU.mult, op1=ALU.add,
                                        accum_out=logit_b[:, ti:ti + 1],
                                    )
                                else:
                                    nc.vector.tensor_tensor(
                                        out=scr[:],
                                        in0=xn_all[:, t * D:(t + 1) * D],
                                        in1=gw[:, 0:D],
                                        op=ALU.mult,
                                    )
                                    nc.vector.tensor_reduce(
                                        out=logit_b[:, ti:ti + 1], in_=scr[:],
                                        axis=AXL.X, op=ALU.add,
                                    )
                            if debug_dump:
                                nc.sync.dma_start(out=dbg["logit"][l, b], in_=logit_b[:])
                            coef_b = tiny_pool.tile([P, BT], dt.float32, tag="coef_b")
                            nc.scalar.activation(
                                out=coef_b[:], in_=logit_b[:], func=AFT.Gelu
                            )
                            if l < NDL - 1:
                                dec_b = tiny_pool.tile([P, BT], dt.float32, tag="dec_b")
                                nc.vector.tensor_scalar(
                                    out=dec_b[:], in0=logit_b[:], scalar1=0.0,
                                    scalar2=None, op0=ALU.is_gt,
                                )
                                nc.vector.scalar_tensor_tensor(
                                    out=rb[:], in0=rb[:], scalar=2.0, in1=dec_b[:],
                                    op0=ALU.mult, op1=ALU.add,
                                )
                        if do_dacc:
                            for ti, gw in enumerate(gws):
                                t = b * BT + ti
                                gout = gw[:, D:WB].bitcast(dt.bfloat16)
                                if l == 0:
                                    dacc = dacc_pool.tile([P, D], dt.bfloat16)
                                    daccs[t] = dacc
                                    nc.vector.tensor_scalar(
                                        out=dacc[:], in0=gout,
                                        scalar1=coef_b[:, ti:ti + 1], scalar2=None,
                                        op0=ALU.mult,
                                    )
                                else:
                                    nc.vector.scalar_tensor_tensor(
                                        out=daccs[t][:], in0=gout,
                                        scalar=coef_b[:, ti:ti + 1],
                                        in1=daccs[t][:],
                                        op0=ALU.mult, op1=ALU.add,
                                    )
                        if l == NDL - 1:
                            for ti in range(BT):
                                _emit_output(
                                    nc, b * BT + ti, mskTs, daccs, woT_sb, ident,
                                    ops_pool, osb_pool, out_d, do_identmm, dt, ALU,
                                )
            else:
                for t in range(NT):
                    _emit_output(
                        nc, t, mskTs, daccs, woT_sb, ident,
                        ops_pool, osb_pool, out_d, deep_on, dt, ALU,
                    )

    nc.compile()
    return nc


def _emit_output(nc, t, mskTs, daccs, woT_sb, ident, ops_pool, osb_pool, out_d,
                 deep_on, dt, ALU):
    mskT = mskTs[t]
    osb = osb_pool.tile([P, D], dt.float32, name="osb")
    for h in range(2):
        ops = ops_pool.tile([P, 512], dt.float32, space="PSUM", name="ops")
        for c in range(2):
            nc.tensor.matmul(
                out=ops[:],
                lhsT=mskT[:, c * P:(c + 1) * P],
                rhs=woT_sb[:, c * D + h * 512: c * D + h * 512 + 512],
                start=(c == 0),
                stop=(c == 1 and not deep_on),
                skip_group_check=True,
            )
        if deep_on:
            nc.tensor.matmul(
                out=ops[:],
                lhsT=ident[:],
                rhs=daccs[t][:, h * 512:(h + 1) * 512],
                start=False,
                stop=True,
                skip_group_check=True,
            )
        nc.scalar.copy(out=osb[:, h * 512:(h + 1) * 512], in_=ops[:])
    nc.sync.dma_start(out=out_d[t * P:(t + 1) * P, :], in_=osb[:])


def host_prep(x, w_in, w_out):
    """Build the per-core input maps (host-side transposes/tilings)."""
    import ml_dtypes

    bf16 = ml_dtypes.bfloat16
    x = np.ascontiguousarray(x, np.float32)
    w_in = np.ascontiguousarray(w_in, np.float32)
    w_out = np.ascontiguousarray(w_out, np.float32)

    w_inT_sh = np.zeros((SHN, D), np.float32)
    w_inT_sh[:SH_NODES] = w_in[:SH_NODES]
    w_inT_sh = np.ascontiguousarray(
        w_inT_sh.T.reshape(KC, P, SHN)
    )  # [k,p,n] = w_in[n, k*128+p]

    woT_bf = np.zeros((SHN, D), np.float32)
    woT_bf[:SH_NODES] = w_out[:, :SH_NODES].T
    woT_bf = np.ascontiguousarray(
        woT_bf.reshape(2, P, D).astype(bf16)
    )  # [c,p,o] = w_out[o, c*128+p]

    # wcat rows: 1024 f32 w_in | 1024 bf16 w_outT packed into 512 f32 words
    wo_bf = np.ascontiguousarray(w_out.T.astype(bf16))   # (4095, 1024) bf16
    wo_packed = np.frombuffer(wo_bf.tobytes(), dtype=np.float32).reshape(N_NODES, D // 2)
    wcat = np.ascontiguousarray(
        np.concatenate([w_in, wo_packed], axis=1)
    )  # (4095, 1536) f32 bytes

    in_maps = []
    for c in range(N_CORES):
        xs = x[c * TOK:(c + 1) * TOK]
        xT = np.ascontiguousarray(
            xs.reshape(NT, P, KC, P).transpose(0, 2, 3, 1)
        )  # [t,k,p,j] = xs[t*128+j, k*128+p]
        in_maps.append(
            {
                "x": np.ascontiguousarray(xs),
                "xT": xT,
                "wcat": wcat,
                "w_inT_sh": w_inT_sh,
                "woT_bf": woT_bf,
            }
        )
    return in_maps


_NC_CACHE = {}


def kernel(x, w_in, w_out, force_depth=None, **_ignored):
    from concourse.bass_utils import run_bass_kernel_spmd

    if "nc" not in _NC_CACHE:
        _NC_CACHE["nc"] = build_nc()
    nc = _NC_CACHE["nc"]

    in_maps = host_prep(np.asarray(x), np.asarray(w_in), np.asarray(w_out))
    res = run_bass_kernel_spmd(nc, in_maps, core_ids=list(range(N_CORES)))
    out = np.concatenate([res.results[c]["out"] for c in range(N_CORES)], axis=0)
    return out.astype(np.float32)


if __name__ == "__main__":
    import reference

    inputs = reference.setup_inputs()
    expected = np.asarray(reference.reference(**inputs))
    actual = kernel(**{k: np.asarray(v) for k, v in inputs.items()})
    err = np.abs(actual - expected).max()
    print("absmax err:", err)
